# revision 17
# baseline (speedup 1.0000x reference)
"""Trainium2 Bass kernel for nn_MultiHeadAttention_83863531421896.

Full-input contract: kernel(**inputs) takes the unsharded tensors and
returns the full (2, 2048, 1024) output. Internally the 16 heads are
sharded 2-per-core across 8 NeuronCores (tensor parallel); each core
computes its heads' attention plus its slice of the output projection,
and the 8 partial projections are reduced on the host.

v3 dataflow per core (heads h0, h1), bf16 operands / fp32 PSUM accum:
  qkvT = W_qkv_slice @ x^T (bf16), m-chunk-outer so matmuls start as
         soon as the first x chunk lands; V^T -> V via PE transposes
         per chunk, V packed [V | ones] per 128-key tile
  attention per (q-chunk, key-tile): S^T both heads -> one 2-bank PSUM
         group; ONE exp over [128,1024] on ScalarE (scale=1/8) -> bf16;
         A^T V with full 128-key stationary [V|ones] -> out + softmax
         denominators accumulated over 16 key tiles in PSUM
  normalization fully on-chip: denom row -> reciprocal_approx_fast,
         partition-broadcast via a rank-1 PE matmul (ones column x
         denom row), multiply (bf16 into outT)
  out-proj (wo^T slice @ outT chunk) matmuls are interleaved one-per-
         key-tile across BOTH batches' attention loops so they fill
         TensorE slack under the ScalarE-bound softmax pipeline;
         partial output stored bf16, reduced across cores on host
"""

import sys

if "/opt/trn_rl_repo" not in sys.path:
    sys.path.insert(0, "/opt/trn_rl_repo")

import numpy as np

B = 2
S = 2048
D = 1024
H = 16
HD = 64
N_CORES = 8
HEADS_PER_CORE = H // N_CORES  # 2
M = B * S                      # 4096 tokens
N_MCHUNK_B = S // 512          # 4 m-chunks of 512 tokens per batch
N_KTILE = D // 128             # 8 contraction tiles for qkv
N_QCHUNK = S // 512            # 4 q-chunks per batch
N_KKTILE = S // 128            # 16 key tiles per batch
SCALE = 1.0 / np.sqrt(HD)

_CACHE = {}


def _build_module():
    import concourse.bass as bass
    import concourse.tile as tile
    from concourse import bacc, mybir

    f32 = mybir.dt.float32
    f32r = mybir.dt.float32r
    bf16 = mybir.dt.bfloat16
    Exp = mybir.ActivationFunctionType.Exp

    nc = bacc.Bacc("TRN2", target_bir_lowering=False, debug=False,
                   num_devices=N_CORES)

    xt_ap = nc.dram_tensor("xt", [D, M], bf16, kind="ExternalInput").ap()
    wqa_ap = nc.dram_tensor("wqa", [D, 128], bf16, kind="ExternalInput").ap()
    wqb_ap = nc.dram_tensor("wqb", [D, 128], bf16, kind="ExternalInput").ap()
    wv_ap = nc.dram_tensor("wv", [D, 128], bf16, kind="ExternalInput").ap()
    wo_ap = nc.dram_tensor("wo", [128, D], bf16, kind="ExternalInput").ap()
    ba_ap = nc.dram_tensor("ba", [128, 1], f32, kind="ExternalInput").ap()
    bb_ap = nc.dram_tensor("bb", [128, 1], f32, kind="ExternalInput").ap()
    bv_ap = nc.dram_tensor("bv", [128, 1], f32, kind="ExternalInput").ap()
    ones_ap = nc.dram_tensor("ones", [128, 64], bf16, kind="ExternalInput").ap()
    ident_ap = nc.dram_tensor("ident", [128, 128], bf16, kind="ExternalInput").ap()
    out_ap = nc.dram_tensor("partial", [D, M], bf16, kind="ExternalOutput").ap()
    sums_dram = nc.dram_tensor(
        "sums_scratch", [B * N_QCHUNK * HEADS_PER_CORE, 512], f32).ap()

    xt3 = xt_ap.rearrange("(k p) m -> p k m", p=128)       # [128, 8, M]
    out3 = out_ap.rearrange("(k p) m -> p k m", p=128)     # [128, 8, M]

    with tile.TileContext(nc) as tc:
        with tc.tile_pool(name="persist", bufs=1) as persist, \
             tc.tile_pool(name="const", bufs=1) as const, \
             tc.tile_pool(name="xpool", bufs=4) as xpool, \
             tc.tile_pool(name="vt_pool", bufs=2) as vt_pool, \
             tc.tile_pool(name="ps8", bufs=1, space="PSUM") as ps8, \
             tc.tile_pool(name="epool", bufs=3) as epool, \
             tc.tile_pool(name="stage", bufs=2) as stage, \
             tc.tile_pool(name="fin", bufs=2) as fin:
            qka_sb = persist.tile([128, M], bf16, tag="qka")
            qkb_sb = persist.tile([128, M], bf16, tag="qkb")
            v_sb = persist.tile([128, B, N_KKTILE, HEADS_PER_CORE, 65], bf16,
                                tag="vsb")
            outt_sb = persist.tile([128, M], bf16, tag="outt")

            ident_sb = const.tile([128, 128], bf16, tag="ident")
            nc.sync.dma_start(ident_sb[:], ident_ap[:])
            wo_sb = const.tile([128, D], bf16, tag="wo")
            nc.gpsimd.dma_start(wo_sb[:], wo_ap[:])
            ba_sb = const.tile([128, 1], f32, tag="ba")
            nc.gpsimd.dma_start(ba_sb[:], ba_ap[:])
            bb_sb = const.tile([128, 1], f32, tag="bb")
            nc.gpsimd.dma_start(bb_sb[:], bb_ap[:])
            bv_sb = const.tile([128, 1], f32, tag="bv")
            nc.gpsimd.dma_start(bv_sb[:], bv_ap[:])
            ones_sb = const.tile([128, 64], bf16, tag="ones")
            nc.gpsimd.dma_start(ones_sb[:], ones_ap[:])
            wq_sb = const.tile([128, 3, N_KTILE, 128], bf16, tag="wq")
            nc.scalar.dma_start(
                wq_sb[:, 0], wqa_ap.rearrange("(k p) c -> p k c", p=128))
            nc.scalar.dma_start(
                wq_sb[:, 1], wqb_ap.rearrange("(k p) c -> p k c", p=128))
            nc.scalar.dma_start(
                wq_sb[:, 2], wv_ap.rearrange("(k p) c -> p k c", p=128))
            nc.gpsimd.dma_start(
                v_sb[:, :, :, :, 64:65],
                ones_ap[:, 0:B * N_KKTILE * HEADS_PER_CORE].rearrange(
                    "p (b t h) -> p b t h", b=B, t=N_KKTILE)[:, :, :, :, None])

            def load_x(b2, engs):
                # each chunk split in two half-loads on different queues
                xss = []
                for mc in range(N_MCHUNK_B):
                    mi = b2 * N_MCHUNK_B + mc
                    xs = xpool.tile([128, N_KTILE, 512], bf16, tag="xs",
                                    name=f"xs{mi}")
                    cols = slice(mi * 512, (mi + 1) * 512)
                    engs[0].dma_start(xs[:, 0:4], xt3[:, 0:4, cols])
                    engs[1].dma_start(xs[:, 4:8], xt3[:, 4:8, cols])
                    xss.append(xs)
                return xss

            # Keep the PE busy while the first x chunks stream in, so the
            # HAM clock gate is released before the real matmuls start.
            def warmup(n):
                wm = ps8.tile([128, 512], f32, tag="sp0", name="wm")
                for _ in range(n):
                    nc.tensor.matmul(wm[0:128, 0:128], ident_sb[:], ident_sb[:],
                                     start=True, stop=True)

            def qkv_phase(b2, xss):
                vt_sb = vt_pool.tile([128, S], bf16, tag="vt", name=f"vt{b2}")
                g = 0
                for mc in range(N_MCHUNK_B):
                    for ei, (bias, dest) in enumerate(
                            [(ba_sb, qka_sb), (bb_sb, qkb_sb), (bv_sb, vt_sb)]):
                        ps = ps8.tile([128, 512], f32, tag=f"sp{g % 2}",
                                      name="qp")
                        g += 1
                        for ki in range(N_KTILE):
                            nc.tensor.matmul(ps[:], wq_sb[:, ei, ki],
                                             xss[mc][:, ki],
                                             start=(ki == 0),
                                             stop=(ki == N_KTILE - 1))
                        col = (b2 * N_MCHUNK_B + mc) if ei < 2 else mc
                        nc.vector.tensor_scalar_add(
                            dest[:, col * 512:(col + 1) * 512], ps[:], bias[:])
                    for kt in range(mc * 4, mc * 4 + 4):
                        tp = ps8.tile([128, 128], bf16, tag=f"sp{g % 2}",
                                      name="tp")
                        g += 1
                        nc.tensor.transpose(
                            tp[:], vt_sb[:, kt * 128:(kt + 1) * 128],
                            ident_sb[:])
                        for h in range(HEADS_PER_CORE):
                            nc.vector.tensor_copy(v_sb[:, b2, kt, h, 0:64],
                                                  tp[:, h * 64:(h + 1) * 64])

            def norm_qi(b2, qi, avp):
                qcol = b2 * S + qi * 512
                for h in range(HEADS_PER_CORE):
                    st = stage.tile([128, 512], f32, tag="st", name="st")
                    nc.vector.tensor_copy(st[0:65, :], avp[h][0:65, :])
                    rr = stage.tile([128, 512], f32, tag="rr", name="rr")
                    nc.gpsimd.dma_start(rr[0:1, :], st[64:65, :])
                    rb = stage.tile([128, 512], f32, tag="rb", name="rb")
                    nc.gpsimd.partition_broadcast(rb[0:64, :], rr[0:1, :])
                    rb2 = stage.tile([128, 512], f32, tag="rb2", name="rb2")
                    nc.vector.reciprocal_approx_fast(rb2[0:64, :], rb[0:64, :])
                    if h == 0:
                        nc.vector.tensor_mul(outt_sb[0:64, qcol:qcol + 512],
                                             st[0:64, :], rb2[0:64, :])
                    else:
                        tm = stage.tile([128, 512], bf16, tag="tm", name="tm")
                        nc.vector.tensor_mul(tm[0:64, :], st[0:64, :],
                                             rb2[0:64, :])
                        nc.gpsimd.dma_start(outt_sb[64:128, qcol:qcol + 512],
                                            tm[0:64, :])

            def op_chunk_makers(bo, mc):
                # 8 closures, each emitting one out-proj matmul (+cast);
                # the last also emits the batched store.
                mrow = bo * S + mc * 512
                state = {}

                def mk(et):
                    def emit():
                        if "fo" not in state:
                            state["fo"] = fin.tile([128, D // 128, 512], bf16,
                                                   tag="fo", name="fo")
                        fp = ps8.tile([128, 512], f32, tag=f"sp{et % 2}",
                                      name="fp")
                        nc.tensor.matmul(fp[:],
                                         wo_sb[:, et * 128:(et + 1) * 128],
                                         outt_sb[:, mrow:mrow + 512],
                                         start=True, stop=True)
                        nc.vector.tensor_copy(state["fo"][:, et], fp[:])
                        if et == D // 128 - 1:
                            eng = nc.sync if mc % 2 == 0 else nc.gpsimd
                            eng.dma_start(
                                out3[:, :, mrow:mrow + 512], state["fo"][:])
                    return emit
                return [mk(et) for et in range(D // 128)]

            def attn_phase(b2, op_lists):
                # flat software pipeline over (qi, kt) slots: AV matmuls lag
                # one slot behind exp, crossing q-chunk boundaries so the
                # ScalarE exp stream never stalls; each chunk's norm is
                # emitted right after its last AV (and before the next
                # chunk's first AV claims the same PSUM tag slots).
                avp = {}
                pending = None
                nops = {}

                def emit_av(qi, kt, es_kt):
                    if kt == 0:
                        avp[qi] = [
                            ps8.tile([128, 512], f32, tag=f"av{h}",
                                     name=f"av{h}")
                            for h in range(HEADS_PER_CORE)]
                    first = (kt == 0)
                    last = (kt == N_KKTILE - 1)
                    for h in range(HEADS_PER_CORE):
                        nc.tensor.matmul(
                            avp[qi][h][0:65, :],
                            v_sb[:, b2, kt, h, :],
                            es_kt[:, h, :],
                            start=first, stop=last)
                    if last:
                        norm_qi(b2, qi, avp.pop(qi))

                for qi in range(N_QCHUNK):
                    qcol = b2 * S + qi * 512
                    op_mms = op_lists[qi]
                    nops[qi] = 0
                    for kt in range(N_KKTILE):
                        kkcol = b2 * S + kt * 128
                        sc = ps8.tile([128, 2, 512], f32, tag=f"sc{kt % 2}",
                                      name=f"sc{kt % 2}")
                        for h in range(HEADS_PER_CORE):
                            nc.tensor.matmul(
                                sc[:, h, :],
                                qkb_sb[h * 64:(h + 1) * 64, kkcol:kkcol + 128],
                                qka_sb[h * 64:(h + 1) * 64, qcol:qcol + 512],
                                start=True, stop=True)
                        es = epool.tile([128, 2, 512], bf16, tag="e",
                                        name="es")
                        nc.scalar.activation(es[:], sc[:], Exp, scale=SCALE)
                        if pending is not None:
                            emit_av(*pending)
                        if nops[qi] < len(op_mms) and kt % 2 == 0:
                            op_mms[nops[qi]]()
                            nops[qi] += 1
                        pending = (qi, kt, es)
                    while nops[qi] < len(op_mms):
                        op_mms[nops[qi]]()
                        nops[qi] += 1
                emit_av(*pending)

            # ---- schedule ----
            xss0 = load_x(0, (nc.sync, nc.gpsimd))
            warmup(24)
            qkv_phase(0, xss0)
            xss1 = load_x(1, (nc.sync, nc.gpsimd))
            attn_phase(0, [[],
                           op_chunk_makers(0, 0),
                           op_chunk_makers(0, 1),
                           op_chunk_makers(0, 2)])
            qkv_phase(1, xss1)
            attn_phase(1, [op_chunk_makers(0, 3),
                           op_chunk_makers(1, 0),
                           op_chunk_makers(1, 1),
                           op_chunk_makers(1, 2)])
            for emit in op_chunk_makers(1, N_QCHUNK - 1):
                emit()
    nc.compile()
    return nc


def _shard_inputs(x, w_qkv, b_qkv, w_out):
    import ml_dtypes

    bf16 = ml_dtypes.bfloat16
    xt = np.ascontiguousarray(x.reshape(M, D).T).astype(bf16)  # (1024, 4096)
    ones = np.ones((128, 64), dtype=bf16)
    ident = np.eye(128, dtype=bf16)
    in_maps = []
    for c in range(N_CORES):
        h0 = HEADS_PER_CORE * c
        rows_q, rows_k, rows_v, dcols = [], [], [], []
        for h in (h0, h0 + 1):
            rows_q += list(range(h * 192, h * 192 + 64))
            rows_k += list(range(h * 192 + 64, h * 192 + 128))
            rows_v += list(range(h * 192 + 128, h * 192 + 192))
            dcols += list(range(h * 64, (h + 1) * 64))
        in_maps.append({
            "xt": xt,
            "wqa": np.ascontiguousarray(w_qkv[rows_q, :].T).astype(bf16),
            "wqb": np.ascontiguousarray(w_qkv[rows_k, :].T).astype(bf16),
            "wv": np.ascontiguousarray(w_qkv[rows_v, :].T).astype(bf16),
            "wo": np.ascontiguousarray(w_out[:, dcols].T).astype(bf16),
            "ba": np.ascontiguousarray(b_qkv[rows_q].reshape(128, 1)),
            "bb": np.ascontiguousarray(b_qkv[rows_k].reshape(128, 1)),
            "bv": np.ascontiguousarray(b_qkv[rows_v].reshape(128, 1)),
            "ones": ones,
            "ident": ident,
        })
    return in_maps


def kernel(x, w_qkv, b_qkv, w_out, b_out, _trace=False):
    from concourse.bass_utils import run_bass_kernel_spmd

    x = np.asarray(x, dtype=np.float32)
    w_qkv = np.asarray(w_qkv, dtype=np.float32)
    b_qkv = np.asarray(b_qkv, dtype=np.float32)
    w_out = np.asarray(w_out, dtype=np.float32)
    b_out = np.asarray(b_out, dtype=np.float32)

    if "nc" not in _CACHE:
        _CACHE["nc"] = _build_module()
    nc = _CACHE["nc"]

    in_maps = _shard_inputs(x, w_qkv, b_qkv, w_out)
    res = run_bass_kernel_spmd(nc, in_maps, list(range(N_CORES)), trace=_trace)
    acc = np.zeros((D, M), dtype=np.float32)
    for c in range(N_CORES):
        acc += res.results[c]["partial"].astype(np.float32)
    acc = acc.T + b_out
    out = acc.astype(np.float32).reshape(B, S, D)
    if _trace:
        _CACHE["last_exec_time_ns"] = res.exec_time_ns
        _CACHE["last_res"] = res
    return out


# revision 18
# speedup vs baseline: 1.0191x; 1.0191x over previous
"""Trainium2 Bass kernel for nn_MultiHeadAttention_83863531421896.

Full-input contract: kernel(**inputs) takes the unsharded tensors and
returns the full (2, 2048, 1024) output. Internally the 16 heads are
sharded 2-per-core across 8 NeuronCores (tensor parallel); each core
computes its heads' attention plus its slice of the output projection,
and the 8 partial projections are reduced on the host.

v3 dataflow per core (heads h0, h1), bf16 operands / fp32 PSUM accum:
  qkvT = W_qkv_slice @ x^T (bf16), m-chunk-outer so matmuls start as
         soon as the first x chunk lands; V^T -> V via PE transposes
         per chunk, V packed [V | ones] per 128-key tile
  attention per (q-chunk, key-tile): S^T both heads -> one 2-bank PSUM
         group; ONE exp over [128,1024] on ScalarE (scale=1/8) -> bf16;
         A^T V with full 128-key stationary [V|ones] -> out + softmax
         denominators accumulated over 16 key tiles in PSUM
  normalization fully on-chip: denom row -> reciprocal_approx_fast,
         partition-broadcast via a rank-1 PE matmul (ones column x
         denom row), multiply (bf16 into outT)
  out-proj (wo^T slice @ outT chunk) matmuls are interleaved one-per-
         key-tile across BOTH batches' attention loops so they fill
         TensorE slack under the ScalarE-bound softmax pipeline;
         partial output stored bf16, reduced across cores on host
"""

import sys

if "/opt/trn_rl_repo" not in sys.path:
    sys.path.insert(0, "/opt/trn_rl_repo")

import numpy as np

B = 2
S = 2048
D = 1024
H = 16
HD = 64
N_CORES = 8
HEADS_PER_CORE = H // N_CORES  # 2
M = B * S                      # 4096 tokens
N_MCHUNK_B = S // 512          # 4 m-chunks of 512 tokens per batch
N_KTILE = D // 128             # 8 contraction tiles for qkv
N_QCHUNK = S // 512            # 4 q-chunks per batch
N_KKTILE = S // 128            # 16 key tiles per batch
SCALE = 1.0 / np.sqrt(HD)

_CACHE = {}


def _build_module():
    import concourse.bass as bass
    import concourse.tile as tile
    from concourse import bacc, mybir

    f32 = mybir.dt.float32
    f32r = mybir.dt.float32r
    bf16 = mybir.dt.bfloat16
    Exp = mybir.ActivationFunctionType.Exp

    nc = bacc.Bacc("TRN2", target_bir_lowering=False, debug=False,
                   num_devices=N_CORES)

    xt_ap = nc.dram_tensor("xt", [D, M], bf16, kind="ExternalInput").ap()
    wqa_ap = nc.dram_tensor("wqa", [D, 128], bf16, kind="ExternalInput").ap()
    wqb_ap = nc.dram_tensor("wqb", [D, 128], bf16, kind="ExternalInput").ap()
    wv_ap = nc.dram_tensor("wv", [D, 128], bf16, kind="ExternalInput").ap()
    wo_ap = nc.dram_tensor("wo", [128, D], bf16, kind="ExternalInput").ap()
    ba_ap = nc.dram_tensor("ba", [128, 1], f32, kind="ExternalInput").ap()
    bb_ap = nc.dram_tensor("bb", [128, 1], f32, kind="ExternalInput").ap()
    bv_ap = nc.dram_tensor("bv", [128, 1], f32, kind="ExternalInput").ap()
    ones_ap = nc.dram_tensor("ones", [128, 64], bf16, kind="ExternalInput").ap()
    ident_ap = nc.dram_tensor("ident", [128, 128], bf16, kind="ExternalInput").ap()
    out_ap = nc.dram_tensor("partial", [D, M], bf16, kind="ExternalOutput").ap()
    sums_dram = nc.dram_tensor(
        "sums_scratch", [B * N_QCHUNK * HEADS_PER_CORE, 512], f32).ap()

    xt3 = xt_ap.rearrange("(k p) m -> p k m", p=128)       # [128, 8, M]
    out3 = out_ap.rearrange("(k p) m -> p k m", p=128)     # [128, 8, M]

    with tile.TileContext(nc) as tc:
        with tc.tile_pool(name="persist", bufs=1) as persist, \
             tc.tile_pool(name="const", bufs=1) as const, \
             tc.tile_pool(name="xpool", bufs=4) as xpool, \
             tc.tile_pool(name="vt_pool", bufs=2) as vt_pool, \
             tc.tile_pool(name="ps8", bufs=1, space="PSUM") as ps8, \
             tc.tile_pool(name="epool", bufs=3) as epool, \
             tc.tile_pool(name="stage", bufs=2) as stage, \
             tc.tile_pool(name="fin", bufs=2) as fin:
            qka_sb = persist.tile([128, M], bf16, tag="qka")
            qkb_sb = persist.tile([128, M], bf16, tag="qkb")
            v_sb = persist.tile([128, B, N_KKTILE, HEADS_PER_CORE, 65], bf16,
                                tag="vsb")
            outt_sb = persist.tile([128, M], bf16, tag="outt")

            ident_sb = const.tile([128, 128], bf16, tag="ident")
            nc.sync.dma_start(ident_sb[:], ident_ap[:])
            wo_sb = const.tile([128, D], bf16, tag="wo")
            nc.gpsimd.dma_start(wo_sb[:], wo_ap[:])
            ba_sb = const.tile([128, 1], f32, tag="ba")
            nc.gpsimd.dma_start(ba_sb[:], ba_ap[:])
            bb_sb = const.tile([128, 1], f32, tag="bb")
            nc.gpsimd.dma_start(bb_sb[:], bb_ap[:])
            bv_sb = const.tile([128, 1], f32, tag="bv")
            nc.gpsimd.dma_start(bv_sb[:], bv_ap[:])
            ones_sb = const.tile([128, 64], bf16, tag="ones")
            nc.gpsimd.dma_start(ones_sb[:], ones_ap[:])
            wq_sb = const.tile([128, 3, N_KTILE, 128], bf16, tag="wq")
            nc.scalar.dma_start(
                wq_sb[:, 0], wqa_ap.rearrange("(k p) c -> p k c", p=128))
            nc.scalar.dma_start(
                wq_sb[:, 1], wqb_ap.rearrange("(k p) c -> p k c", p=128))
            nc.scalar.dma_start(
                wq_sb[:, 2], wv_ap.rearrange("(k p) c -> p k c", p=128))
            nc.gpsimd.dma_start(
                v_sb[:, :, :, :, 64:65],
                ones_ap[:, 0:B * N_KKTILE * HEADS_PER_CORE].rearrange(
                    "p (b t h) -> p b t h", b=B, t=N_KKTILE)[:, :, :, :, None])

            def load_x(b2, engs):
                # each chunk split in two half-loads on different queues
                xss = []
                for mc in range(N_MCHUNK_B):
                    mi = b2 * N_MCHUNK_B + mc
                    xs = xpool.tile([128, N_KTILE, 512], bf16, tag="xs",
                                    name=f"xs{mi}")
                    cols = slice(mi * 512, (mi + 1) * 512)
                    engs[0].dma_start(xs[:, 0:4], xt3[:, 0:4, cols])
                    engs[1].dma_start(xs[:, 4:8], xt3[:, 4:8, cols])
                    xss.append(xs)
                return xss

            # Keep the PE busy while the first x chunks stream in, so the
            # HAM clock gate is released before the real matmuls start.
            def warmup(n):
                wm = ps8.tile([128, 512], f32, tag="sp0", name="wm")
                for _ in range(n):
                    nc.tensor.matmul(wm[0:128, 0:128], ident_sb[:], ident_sb[:],
                                     start=True, stop=True)

            def qkv_phase(b2, xss):
                vt_sb = vt_pool.tile([128, S], bf16, tag="vt", name=f"vt{b2}")
                g = 0
                for mc in range(N_MCHUNK_B):
                    for ei, (bias, dest) in enumerate(
                            [(ba_sb, qka_sb), (bb_sb, qkb_sb), (bv_sb, vt_sb)]):
                        ps = ps8.tile([128, 512], f32, tag=f"sp{g % 2}",
                                      name="qp")
                        g += 1
                        for ki in range(N_KTILE):
                            nc.tensor.matmul(ps[:], wq_sb[:, ei, ki],
                                             xss[mc][:, ki],
                                             start=(ki == 0),
                                             stop=(ki == N_KTILE - 1))
                        col = (b2 * N_MCHUNK_B + mc) if ei < 2 else mc
                        nc.vector.tensor_scalar_add(
                            dest[:, col * 512:(col + 1) * 512], ps[:], bias[:])
                    for kt in range(mc * 4, mc * 4 + 4):
                        tp = ps8.tile([128, 128], bf16, tag=f"sp{g % 2}",
                                      name="tp")
                        g += 1
                        nc.tensor.transpose(
                            tp[:], vt_sb[:, kt * 128:(kt + 1) * 128],
                            ident_sb[:])
                        for h in range(HEADS_PER_CORE):
                            nc.vector.tensor_copy(v_sb[:, b2, kt, h, 0:64],
                                                  tp[:, h * 64:(h + 1) * 64])

            def norm_qi(b2, qi, avp):
                qcol = b2 * S + qi * 512
                for h in range(HEADS_PER_CORE):
                    st = stage.tile([128, 512], f32, tag="st", name="st")
                    nc.vector.tensor_copy(st[0:65, :], avp[h][0:65, :])
                    rr = stage.tile([128, 512], f32, tag="rr", name="rr")
                    nc.gpsimd.dma_start(rr[0:1, :], st[64:65, :])
                    rb = stage.tile([128, 512], f32, tag="rb", name="rb")
                    nc.gpsimd.partition_broadcast(rb[0:64, :], rr[0:1, :])
                    rb2 = stage.tile([128, 512], f32, tag="rb2", name="rb2")
                    nc.vector.reciprocal_approx_fast(rb2[0:64, :], rb[0:64, :])
                    if h == 0:
                        nc.vector.tensor_mul(outt_sb[0:64, qcol:qcol + 512],
                                             st[0:64, :], rb2[0:64, :])
                    else:
                        tm = stage.tile([128, 512], bf16, tag="tm", name="tm")
                        nc.vector.tensor_mul(tm[0:64, :], st[0:64, :],
                                             rb2[0:64, :])
                        nc.gpsimd.dma_start(outt_sb[64:128, qcol:qcol + 512],
                                            tm[0:64, :])

            def op_chunk_makers(bo, mc):
                # 8 closures, each emitting one out-proj matmul (+cast);
                # the last also emits the batched store.
                mrow = bo * S + mc * 512
                state = {}

                def mk(et):
                    def emit():
                        if "fo" not in state:
                            state["fo"] = fin.tile([128, D // 128, 512], bf16,
                                                   tag="fo", name="fo")
                        fp = ps8.tile([128, 512], f32, tag=f"sp{et % 2}",
                                      name="fp")
                        nc.tensor.matmul(fp[:],
                                         wo_sb[:, et * 128:(et + 1) * 128],
                                         outt_sb[:, mrow:mrow + 512],
                                         start=True, stop=True)
                        nc.vector.tensor_copy(state["fo"][:, et], fp[:])
                        if et == D // 128 - 1:
                            eng = nc.sync if mc % 2 == 0 else nc.gpsimd
                            eng.dma_start(
                                out3[:, :, mrow:mrow + 512], state["fo"][:])
                    return emit
                return [mk(et) for et in range(D // 128)]

            def attn_phase(b2, op_lists):
                # flat software pipeline over (qi, kt) slots: AV matmuls lag
                # one slot behind exp, crossing q-chunk boundaries so the
                # ScalarE exp stream never stalls; each chunk's norm is
                # emitted right after its last AV (and before the next
                # chunk's first AV claims the same PSUM tag slots).
                avp = {}
                pending = None
                nops = {}

                def emit_av(qi, kt, es_kt):
                    if kt == 0:
                        avp[qi] = [
                            ps8.tile([128, 512], f32, tag=f"av{h}",
                                     name=f"av{h}")
                            for h in range(HEADS_PER_CORE)]
                    first = (kt == 0)
                    last = (kt == N_KKTILE - 1)
                    for h in range(HEADS_PER_CORE):
                        nc.tensor.matmul(
                            avp[qi][h][0:65, :],
                            v_sb[:, b2, kt, h, :],
                            es_kt[:, h, :],
                            start=first, stop=last)
                    if last:
                        norm_qi(b2, qi, avp.pop(qi))

                for qi in range(N_QCHUNK):
                    qcol = b2 * S + qi * 512
                    op_mms = op_lists[qi]
                    nops[qi] = 0
                    for kt in range(N_KKTILE):
                        kkcol = b2 * S + kt * 128
                        sc = ps8.tile([128, 2, 512], f32, tag=f"sc{kt % 2}",
                                      name=f"sc{kt % 2}")
                        for h in range(HEADS_PER_CORE):
                            nc.tensor.matmul(
                                sc[:, h, :],
                                qkb_sb[h * 64:(h + 1) * 64, kkcol:kkcol + 128],
                                qka_sb[h * 64:(h + 1) * 64, qcol:qcol + 512],
                                start=True, stop=True)
                        es = epool.tile([128, 2, 512], bf16, tag="e",
                                        name="es")
                        nc.scalar.activation(es[:], sc[:], Exp, scale=SCALE)
                        if pending is not None:
                            emit_av(*pending)
                        if nops[qi] < len(op_mms) and kt % 2 == 0:
                            op_mms[nops[qi]]()
                            nops[qi] += 1
                        pending = (qi, kt, es)
                    while nops[qi] < len(op_mms):
                        op_mms[nops[qi]]()
                        nops[qi] += 1
                emit_av(*pending)

            # ---- schedule ----
            xss0 = load_x(0, (nc.sync, nc.gpsimd))
            warmup(40)
            qkv_phase(0, xss0)
            xss1 = load_x(1, (nc.sync, nc.gpsimd))
            attn_phase(0, [[],
                           op_chunk_makers(0, 0),
                           op_chunk_makers(0, 1),
                           op_chunk_makers(0, 2)])
            tc.no_sync_barrier()
            qkv_phase(1, xss1)
            attn_phase(1, [op_chunk_makers(0, 3),
                           op_chunk_makers(1, 0),
                           op_chunk_makers(1, 1),
                           op_chunk_makers(1, 2)])
            for emit in op_chunk_makers(1, N_QCHUNK - 1):
                emit()
    nc.compile()
    return nc


def _shard_inputs(x, w_qkv, b_qkv, w_out):
    import ml_dtypes

    bf16 = ml_dtypes.bfloat16
    xt = np.ascontiguousarray(x.reshape(M, D).T).astype(bf16)  # (1024, 4096)
    ones = np.ones((128, 64), dtype=bf16)
    ident = np.eye(128, dtype=bf16)
    in_maps = []
    for c in range(N_CORES):
        h0 = HEADS_PER_CORE * c
        rows_q, rows_k, rows_v, dcols = [], [], [], []
        for h in (h0, h0 + 1):
            rows_q += list(range(h * 192, h * 192 + 64))
            rows_k += list(range(h * 192 + 64, h * 192 + 128))
            rows_v += list(range(h * 192 + 128, h * 192 + 192))
            dcols += list(range(h * 64, (h + 1) * 64))
        in_maps.append({
            "xt": xt,
            "wqa": np.ascontiguousarray(w_qkv[rows_q, :].T).astype(bf16),
            "wqb": np.ascontiguousarray(w_qkv[rows_k, :].T).astype(bf16),
            "wv": np.ascontiguousarray(w_qkv[rows_v, :].T).astype(bf16),
            "wo": np.ascontiguousarray(w_out[:, dcols].T).astype(bf16),
            "ba": np.ascontiguousarray(b_qkv[rows_q].reshape(128, 1)),
            "bb": np.ascontiguousarray(b_qkv[rows_k].reshape(128, 1)),
            "bv": np.ascontiguousarray(b_qkv[rows_v].reshape(128, 1)),
            "ones": ones,
            "ident": ident,
        })
    return in_maps


def kernel(x, w_qkv, b_qkv, w_out, b_out, _trace=False):
    from concourse.bass_utils import run_bass_kernel_spmd

    x = np.asarray(x, dtype=np.float32)
    w_qkv = np.asarray(w_qkv, dtype=np.float32)
    b_qkv = np.asarray(b_qkv, dtype=np.float32)
    w_out = np.asarray(w_out, dtype=np.float32)
    b_out = np.asarray(b_out, dtype=np.float32)

    if "nc" not in _CACHE:
        _CACHE["nc"] = _build_module()
    nc = _CACHE["nc"]

    in_maps = _shard_inputs(x, w_qkv, b_qkv, w_out)
    res = run_bass_kernel_spmd(nc, in_maps, list(range(N_CORES)), trace=_trace)
    acc = np.zeros((D, M), dtype=np.float32)
    for c in range(N_CORES):
        acc += res.results[c]["partial"].astype(np.float32)
    acc = acc.T + b_out
    out = acc.astype(np.float32).reshape(B, S, D)
    if _trace:
        _CACHE["last_exec_time_ns"] = res.exec_time_ns
        _CACHE["last_res"] = res
    return out
